# revision 1
# baseline (speedup 1.0000x reference)
# DGCNN graph-feature module on 8 Trainium2 NeuronCores.
#
# Data-parallel over batch B=8 (one batch element per core). Per core:
#   - distance ranking scores nd[i,j] = p_i.p_j - |p_j|^2/2 via PE matmul
#     (65-row contraction with a fused -|p|^2/2 row; per-row monotone in -d)
#   - top-16 neighbors per row via DVE max8 / max_index / match_replace
#   - edge-MLP folded into point space: h1[n,k] = G[:, idx[n,k]] + Cc[:, n]
#     with G = W1a @ p^T, Cc = (W1b - W1a) @ p^T  (W1 = [W1a | W1b])
#   - exact BatchNorm batch stats across all 8 cores via tiny AllReduces of
#     per-channel (sum, sumsq)
#   - max-pool over K commutes with BN2+ReLU because the BN scale
#     gamma2 * rsqrt(var+eps) is positive (gamma2 == 1 in this problem)
import numpy as np
from contextlib import ExitStack

import concourse.bass as bass
from concourse import bacc, library_config
import concourse.tile as tile
from concourse import mybir
from concourse.bass_utils import run_bass_kernel_spmd

B, N, C, K = 8, 4096, 64, 16
NB = N // 128                      # 32 row-blocks of 128 points
M_TOTAL = float(B * N * K)         # BN sample count over the whole batch
EPS = 1e-5
NEG_BIG = -1e30
F32 = mybir.dt.float32
I16 = mybir.dt.int16
U32 = mybir.dt.uint32
AF = mybir.ActivationFunctionType
ALU = mybir.AluOpType
AX = mybir.AxisListType

_NC_CACHE = {}


def build_nc(n_cores=8, use_collectives=True, use_gather=True, stages='ABCDE', rep=1):
    nc = bacc.Bacc("TRN2", target_bir_lowering=False, debug=False, num_devices=n_cores)
    pts = nc.declare_dram_parameter("pts", [N, C], F32, isOutput=False)
    w1aT = nc.declare_dram_parameter("w1aT", [C, C], F32, isOutput=False)
    w1cT = nc.declare_dram_parameter("w1cT", [C, C], F32, isOutput=False)
    w2T = nc.declare_dram_parameter("w2T", [C, C], F32, isOutput=False)
    gb = nc.declare_dram_parameter("gb", [C, 4], F32, isOutput=False)
    out_t = nc.declare_dram_parameter("out_t", [C, N], F32, isOutput=True)

    group = [list(range(n_cores))]

    with tile.TileContext(nc) as tc:
      for _rep in range(rep):
       with ExitStack() as ctx:
        per = ctx.enter_context(tc.tile_pool(name="per", bufs=1))
        small = ctx.enter_context(tc.tile_pool(name="small", bufs=3))
        dram = ctx.enter_context(tc.tile_pool(name="dram", bufs=1, space="DRAM"))

        # ---- persistent SBUF tensors
        pT_lhs = per.tile([C + 1, N], F32)   # rows 0..63 = p^T, row 64 = ones
        pT_rhs = per.tile([C + 1, N], F32)   # rows 0..63 = p^T, row 64 = -|p|^2/2
        GT = per.tile([C, N], F32)
        CcT = per.tile([C, N], F32)
        idxw = per.tile([C, N], I16)         # wrapped+replicated idx, per block cols
        s1cols = per.tile([C, NB], F32)
        q1cols = per.tile([C, NB], F32)
        s2cols = per.tile([C, 4 * NB], F32)
        q2cols = per.tile([C, NB], F32)
        pooledT = per.tile([C, N], F32)

        # ---- DRAM spill + collective bounce buffers
        h1sp = dram.tile([C, N * K], F32)
        cc1_in = dram.tile([C, 2], F32)
        cc1_out = dram.tile([C, 2], F32)
        cc2_in = dram.tile([C, 2], F32)
        cc2_out = dram.tile([C, 2], F32)

        # ---- constants
        identity = per.tile([128, 128], F32)
        ones128 = per.tile([128, 128], F32)
        nc.vector.memset(ones128, 1.0)
        nc.gpsimd.affine_select(
            identity, ones128, pattern=[[-1, 128]], compare_op=ALU.is_equal,
            fill=0.0, base=0, channel_multiplier=1,
        )
        # repmat[p, i] = 1 if i % 16 == p else 0   (shape [16, 4, 16])
        rep = per.tile([16, 4, 16], F32)
        nc.gpsimd.affine_select(
            rep, ones128[0:16, 0:64].rearrange("p (a b) -> p a b", b=16),
            pattern=[[0, 4], [-1, 16]], compare_op=ALU.is_equal,
            fill=0.0, base=0, channel_multiplier=1,
        )
        ones_col = per.tile([C, 1], F32)
        nc.vector.memset(ones_col, 1.0)
        eps_col = per.tile([C, 1], F32)
        nc.vector.memset(eps_col, EPS)

        w1aT_s = per.tile([C, C], F32)
        nc.sync.dma_start(out=w1aT_s, in_=w1aT[:, :])
        w1cT_s = per.tile([C, C], F32)
        nc.sync.dma_start(out=w1cT_s, in_=w1cT[:, :])
        w2T_s = per.tile([C, C], F32)
        nc.sync.dma_start(out=w2T_s, in_=w2T[:, :])
        gb_s = per.tile([C, 4], F32)
        nc.sync.dma_start(out=gb_s, in_=gb[:, :])

        # ================= PHASE A: transpose points, norms, G/Cc =========
        with tc.tile_pool(name="psA", bufs=2, space="PSUM") as psA, \
             tc.tile_pool(name="ldA", bufs=3) as ldA, \
             tc.tile_pool(name="sqA", bufs=1) as sqA:
            for t in range(NB):
                sl = slice(t * 128, (t + 1) * 128)
                pt_tile = ldA.tile([128, C], F32)
                nc.sync.dma_start(out=pt_tile, in_=pts[sl, :])
                ps_tr = psA.tile([C, 128], F32)
                nc.tensor.transpose(ps_tr, pt_tile, identity)
                nc.scalar.activation(pT_lhs[0:C, sl], ps_tr, AF.Copy)
                nc.vector.tensor_copy(pT_rhs[0:C, sl], ps_tr)
            nc.vector.memset(pT_lhs[C:C + 1, :], 1.0)

            sq64 = sqA.tile([C, N], F32, tag="sq64")
            nc.vector.tensor_mul(sq64, pT_rhs[0:C, :], pT_rhs[0:C, :])
            for j in range(N // 512):
                js = slice(j * 512, (j + 1) * 512)
                ps_row = psA.tile([1, 512], F32)
                nc.tensor.matmul(ps_row, lhsT=ones_col, rhs=sq64[:, js],
                                 start=True, stop=True)
                nc.scalar.activation(pT_rhs[C:C + 1, js], ps_row, AF.Copy,
                                     scale=-0.5)
            for j in range(N // 512):
                js = slice(j * 512, (j + 1) * 512)
                ps_g = psA.tile([C, 512], F32)
                nc.tensor.matmul(ps_g, lhsT=w1aT_s, rhs=pT_rhs[0:C, js],
                                 start=True, stop=True)
                nc.scalar.activation(GT[:, js], ps_g, AF.Copy)
                ps_c = psA.tile([C, 512], F32)
                nc.tensor.matmul(ps_c, lhsT=w1cT_s, rhs=pT_rhs[0:C, js],
                                 start=True, stop=True)
                nc.vector.tensor_copy(CcT[:, js], ps_c)

        if use_gather and 'B' in stages:
            nc.gpsimd.load_library(library_config.ap_gather)

        # ================= PHASE B: distances, top-16, gather, h1, stats1 =
        if 'B' not in stages:
            nc.vector.memset(s1cols, 1.0)
            nc.vector.memset(q1cols, 2.0)
        if 'D' not in stages:
            nc.vector.memset(s2cols, 1.0)
            nc.vector.memset(q2cols, 2.0)
            nc.vector.memset(pooledT, 0.5)
        with tc.tile_pool(name="psB", bufs=4, space="PSUM") as psB, \
             tc.tile_pool(name="psBs", bufs=2, space="PSUM") as psBs, \
             tc.tile_pool(name="ndb", bufs=2) as ndb, \
             tc.tile_pool(name="ndb2", bufs=1) as ndb2, \
             tc.tile_pool(name="ghb", bufs=2) as ghb, \
             tc.tile_pool(name="scrb", bufs=1) as scrb:
            for t in range(NB if 'B' in stages else 0):
                sl = slice(t * 128, (t + 1) * 128)
                nd = ndb.tile([128, N], F32)
                for j in range(N // 512):
                    js = slice(j * 512, (j + 1) * 512)
                    ps_nd = psB.tile([128, 512], F32)
                    nc.tensor.matmul(ps_nd, lhsT=pT_lhs[:, sl],
                                     rhs=pT_rhs[:, js], start=True, stop=True)
                    nc.scalar.activation(nd[:, js], ps_nd, AF.Copy)

                m8a = small.tile([128, 8], F32)
                nc.vector.max(out=m8a, in_=nd)
                i8a = small.tile([128, 8], U32)
                nc.vector.max_index(out=i8a, in_max=m8a, in_values=nd)
                ndp = ndb2.tile([128, N], F32)
                nc.vector.match_replace(out=ndp, in_to_replace=m8a,
                                        in_values=nd, imm_value=NEG_BIG)
                m8b = small.tile([128, 8], F32)
                nc.vector.max(out=m8b, in_=ndp)
                i8b = small.tile([128, 8], U32)
                nc.vector.max_index(out=i8b, in_max=m8b, in_values=ndp)

                idxf = small.tile([128, 16], F32)
                nc.vector.tensor_copy(idxf[:, 0:8], i8a)
                nc.vector.tensor_copy(idxf[:, 8:16], i8b)
                ps_tr16 = psBs.tile([16, 128], F32)
                nc.tensor.transpose(ps_tr16, idxf, identity)
                idxTf = small.tile([16, 128], F32)
                nc.scalar.activation(idxTf, ps_tr16, AF.Copy)
                ps_rep = psBs.tile([C, 128], F32)
                nc.tensor.matmul(ps_rep, lhsT=rep, rhs=idxTf,
                                 start=True, stop=True)
                nc.vector.tensor_copy(idxw[:, sl], ps_rep)

                gh = ghb.tile([C, 128, K], F32)
                if use_gather:
                    nc.gpsimd.ap_gather(
                        out_ap=gh, in_ap=GT, idxs_ap=idxw[:, sl],
                        channels=C, num_elems=N, d=1, num_idxs=128 * K,
                    )
                else:
                    nc.vector.memset(gh, 0.0)
                ccb = CcT[:, sl].rearrange("c (n o) -> c n o", o=1).to_broadcast(
                    [C, 128, K])
                h1 = ghb.tile([C, 128, K], F32, tag="h1")
                nc.vector.tensor_add(h1, gh, ccb)
                nc.vector.reduce_sum(out=s1cols[:, t:t + 1],
                                     in_=h1.rearrange("c n k -> c (n k)"),
                                     axis=AX.X)
                h1f = h1.rearrange("c n k -> c (n k)")
                h1sq = scrb.tile([C, 128 * K], F32, tag="h1sq")
                nc.scalar.activation(h1sq, h1f, AF.Square,
                                     accum_out=q1cols[:, t:t + 1])
                nc.sync.dma_start(
                    out=h1sp[:, t * 128 * K:(t + 1) * 128 * K], in_=h1f)

        # ================= PHASE C: stats1 allreduce -> a1, b1 ============
        s1 = small.tile([C, 1], F32)
        nc.vector.reduce_sum(out=s1, in_=s1cols, axis=AX.X)
        q1 = small.tile([C, 1], F32)
        nc.vector.reduce_sum(out=q1, in_=q1cols, axis=AX.X)
        sq1 = small.tile([C, 2], F32)
        nc.vector.tensor_copy(sq1[:, 0:1], s1)
        nc.vector.tensor_copy(sq1[:, 1:2], q1)
        nc.sync.dma_start(out=cc1_in[:], in_=sq1)
        if use_collectives:
            nc.gpsimd.collective_compute(
                "AllReduce", ALU.add, replica_groups=group,
                ins=[cc1_in[:].opt()], outs=[cc1_out[:].opt()],
            )
        else:
            nc.sync.dma_start(out=cc1_out[:], in_=cc1_in[:])
        st1 = small.tile([C, 2], F32)
        nc.sync.dma_start(out=st1, in_=cc1_out[:])

        def stats_to_affine(st, g_col, b_col):
            mean = small.tile([C, 1], F32, tag="mean")
            nc.vector.tensor_scalar_mul(mean, st[:, 0:1], 1.0 / M_TOTAL)
            ex2 = small.tile([C, 1], F32, tag="ex2")
            nc.vector.tensor_scalar_mul(ex2, st[:, 1:2], 1.0 / M_TOTAL)
            m2 = small.tile([C, 1], F32, tag="m2")
            nc.vector.tensor_mul(m2, mean, mean)
            var = small.tile([C, 1], F32, tag="var")
            nc.vector.tensor_sub(var, ex2, m2)
            sd = small.tile([C, 1], F32, tag="sd")
            nc.scalar.activation(sd, var, AF.Sqrt, bias=eps_col)
            rs = small.tile([C, 1], F32, tag="rs")
            nc.vector.reciprocal(rs, sd)
            a = small.tile([C, 1], F32, tag="a_aff")
            nc.vector.tensor_mul(a, g_col, rs)
            tmp = small.tile([C, 1], F32, tag="tmp_aff")
            nc.vector.tensor_mul(tmp, mean, a)
            b = small.tile([C, 1], F32, tag="b_aff")
            nc.vector.tensor_sub(b, b_col, tmp)
            return a, b

        a1, b1 = stats_to_affine(st1, gb_s[:, 0:1], gb_s[:, 1:2])

        # ================= PHASE D: z=relu(a1*h1+b1), h2=W2 z, pool, stats2
        with tc.tile_pool(name="psD", bufs=4, space="PSUM") as psD, \
             tc.tile_pool(name="zb", bufs=2) as zb, \
             tc.tile_pool(name="zscr", bufs=1) as zscr:
            for t in range(NB if 'D' in stages else 0):
                h1t = zb.tile([C, 128 * K], F32, tag="h1t")
                nc.sync.dma_start(
                    out=h1t, in_=h1sp[:, t * 128 * K:(t + 1) * 128 * K])
                z = zb.tile([C, 128 * K], F32, tag="z")
                nc.scalar.activation(z, h1t, AF.Relu, scale=a1, bias=b1)
                h2s = zb.tile([C, 128 * K], F32, tag="h2s")
                for j in range(128 * K // 512):
                    js = slice(j * 512, (j + 1) * 512)
                    ps_h2 = psD.tile([C, 512], F32)
                    nc.tensor.matmul(ps_h2, lhsT=w2T_s, rhs=z[:, js],
                                     start=True, stop=True)
                    nc.scalar.activation(h2s[:, js], ps_h2, AF.Copy,
                                         accum_out=s2cols[:, 4 * t + j:
                                                          4 * t + j + 1])
                h2sq = zscr.tile([C, 128 * K], F32, tag="h2sq")
                nc.scalar.activation(h2sq, h2s, AF.Square,
                                     accum_out=q2cols[:, t:t + 1])
                nc.vector.reduce_max(
                    out=pooledT[:, t * 128:(t + 1) * 128],
                    in_=h2s.rearrange("c (n k) -> c n k", k=K), axis=AX.X)

        # ================= PHASE E: stats2 allreduce -> final =============
        s2 = small.tile([C, 1], F32)
        nc.vector.reduce_sum(out=s2, in_=s2cols, axis=AX.X)
        q2 = small.tile([C, 1], F32)
        nc.vector.reduce_sum(out=q2, in_=q2cols, axis=AX.X)
        sq2 = small.tile([C, 2], F32)
        nc.vector.tensor_copy(sq2[:, 0:1], s2)
        nc.vector.tensor_copy(sq2[:, 1:2], q2)
        nc.sync.dma_start(out=cc2_in[:], in_=sq2)
        if use_collectives:
            nc.gpsimd.collective_compute(
                "AllReduce", ALU.add, replica_groups=group,
                ins=[cc2_in[:].opt()], outs=[cc2_out[:].opt()],
            )
        else:
            nc.sync.dma_start(out=cc2_out[:], in_=cc2_in[:])
        st2 = small.tile([C, 2], F32)
        nc.sync.dma_start(out=st2, in_=cc2_out[:])
        a2, b2 = stats_to_affine(st2, gb_s[:, 2:3], gb_s[:, 3:4])

        nc.scalar.activation(pooledT, pooledT, AF.Relu, scale=a2, bias=b2)
        nc.sync.dma_start(out=out_t[:, :], in_=pooledT)

    nc.finalize()
    return nc


def _get_nc(n_cores=8):
    if n_cores not in _NC_CACHE:
        _NC_CACHE[n_cores] = build_nc(n_cores)
    return _NC_CACHE[n_cores]


def make_in_maps(points, W1, gamma1, beta1, W2, gamma2, beta2, n_cores=8):
    pts = np.ascontiguousarray(np.asarray(points, np.float32))
    W1 = np.asarray(W1, np.float32)
    w1aT = np.ascontiguousarray(W1[:, :C].T)
    w1cT = np.ascontiguousarray((W1[:, C:] - W1[:, :C]).T)
    w2T = np.ascontiguousarray(np.asarray(W2, np.float32).T)
    gbm = np.ascontiguousarray(
        np.stack([np.asarray(gamma1, np.float32), np.asarray(beta1, np.float32),
                  np.asarray(gamma2, np.float32), np.asarray(beta2, np.float32)],
                 axis=1))
    return [
        {"pts": np.ascontiguousarray(pts[b]), "w1aT": w1aT, "w1cT": w1cT,
         "w2T": w2T, "gb": gbm}
        for b in range(n_cores)
    ]


def kernel(points, W1, gamma1, beta1, W2, gamma2, beta2, **run_kwargs):
    nc = _get_nc(B)
    in_maps = make_in_maps(points, W1, gamma1, beta1, W2, gamma2, beta2, B)
    res = run_bass_kernel_spmd(nc, in_maps, core_ids=list(range(B)), **run_kwargs)
    out = np.stack([np.asarray(res.results[b]["out_t"]).T for b in range(B)],
                   axis=0)
    kernel.last_results = res
    return out.astype(np.float32)

